# revision 40
# baseline (speedup 1.0000x reference)
"""NT-Xent (SimCLR) contrastive loss on 8 Trainium2 NeuronCores.

Strategy: data-parallel over rows of z = concat(normalize(x_i), normalize(x_j)).
Host passes each core the full feature-major matrix XT = z_raw.T [512, 8192]
in bf16, column-rotated per core so the identical SPMD program finds its
1024-row slab at columns 0:1024 and the positive partners at columns
4096:5120. Each core:
  - normalizes all of z on-chip (bf16 squares -> PE ones-colsum -> ln/exp
    -> PE broadcast -> scale) quantizing z to fp8e4,
  - computes its [1024, 8192] slab of sim = z @ z.T as fp8 DoubleRow
    matmuls (2 contraction tiles per pass, 2 cols/cycle) into PSUM,
  - fuses exp(sim/T) + row-sum via ScalarE activation accum_out,
  - computes positives as colsums of z_slab * z_partner,
  - reduces to two scalars (sum ln(denom), sum pos); host combines 8 partials.
No collectives: row sums are fully local; the final mean is the gather step.
"""
import math

import numpy as np

N_CORES = 8
B, D = 4096, 512
N = 2 * B              # 8192 rows of z
SLAB = N // N_CORES    # 1024 rows per core
CHUNK = 1024           # normalize-prologue column chunk
TEMP = 0.5
INV_T = 1.0 / TEMP
E_DIAG = math.exp(INV_T)   # exp(sim_ii / T) with sim_ii == 1 after normalize

# which engine squares each contraction tile: True -> gpsimd (Pool), False -> DVE
SQ_ON_POOL = (True, True, True, True)
# row-tiles per chunk whose exp+rowsum run as a DVE Schraudolph bit-trick
# (exp(y) ~ bitcast_f32(int32(y*2^23/ln2 + 127*2^23))) + Pool reduce,
# instead of the ScalarE activation
DVE_EXP_MT = ()
A_SCH = 8388608.0 / math.log(2.0) * INV_T   # 2^23/ln2 * (1/T)
B_SCH = 127.0 * 8388608.0

_CACHE = {}


def _build_nc(loop_k=None):
    """Build the SPMD Bass program (identical for all cores)."""
    import concourse.tile as tile
    from concourse import bacc, mybir

    F32 = mybir.dt.float32
    F32R = mybir.dt.float32r
    BF16 = mybir.dt.bfloat16
    F8 = mybir.dt.float8e4
    I32 = mybir.dt.int32
    EXPF = mybir.ActivationFunctionType.Exp
    LNF = mybir.ActivationFunctionType.Ln
    DR = mybir.MatmulPerfMode.DoubleRow
    KT = D // 128      # 4 contraction tiles
    NCH = N // CHUNK   # 8 prologue chunks
    MT = SLAB // 128   # 8 output row tiles
    NG = N // CHUNK    # 8 column groups in the main loop

    nc = bacc.Bacc("TRN2", target_bir_lowering=False, debug=False)
    xt_d = nc.dram_tensor("xt", [D, N], BF16, kind="ExternalInput")
    ones_d = nc.dram_tensor("ones", [128, 1], F32, kind="ExternalInput")
    ones1_d = nc.dram_tensor("ones1", [1, 128], F32, kind="ExternalInput")
    onesb_d = nc.dram_tensor("onesb", [128, 1], BF16, kind="ExternalInput")
    out_d = nc.dram_tensor("out", [1, 2], F32, kind="ExternalOutput")
    xtr = xt_d.rearrange("(kt p) n -> p kt n", p=128)

    with tile.TileContext(nc) as tc:
        with (
            tc.tile_pool(name="ztp", bufs=2) as ztp,
            tc.tile_pool(name="stream", bufs=2) as stream,
            tc.tile_pool(name="sqp", bufs=2) as sqp,
            tc.tile_pool(name="rows", bufs=2) as rows,
            tc.tile_pool(name="expo", bufs=3) as expo,
            tc.tile_pool(name="eip", bufs=2) as eip,
            tc.tile_pool(name="misc", bufs=2) as misc,
            tc.tile_pool(name="ps_main", bufs=3, space="PSUM") as ps_main,
            tc.tile_pool(name="ps_cs", bufs=2, space="PSUM") as ps_cs,
        ):

            def body(_iv=None):
                ones = misc.tile([128, 1], F32R)
                nc.sync.dma_start(out=ones, in_=ones_d[:, :].bitcast(F32R))
                ones1 = misc.tile([1, 128], F32R)
                nc.sync.dma_start(out=ones1, in_=ones1_d[:, :].bitcast(F32R))
                onesb = misc.tile([128, 1], BF16)
                nc.sync.dma_start(out=onesb, in_=onesb_d[:, :])

                zt = ztp.tile([128, KT, N], F8)
                dparts = misc.tile([128, MT * NG], F32)

                def normalize(c):
                    """Fill zt columns of chunk c with fp8 unit-normalized z."""
                    c0 = c * CHUNK
                    xc = stream.tile([128, KT, CHUNK], BF16, tag="xc")
                    for kt in range(KT):
                        nc.sync.dma_start(
                            out=xc[:, kt, :], in_=xtr[:, kt, c0 : c0 + CHUNK]
                        )
                    sq = sqp.tile([128, KT, CHUNK], BF16, tag="sq")
                    for kt in range(KT):
                        # chunks 0/1 are on the pipeline-priming critical
                        # path: split their squares DVE/Pool to halve latency
                        on_pool = SQ_ON_POOL[kt] if c >= 2 else (kt >= 2)
                        eng = nc.gpsimd if on_pool else nc.vector
                        eng.tensor_mul(
                            out=sq[:, kt, :], in0=xc[:, kt, :], in1=xc[:, kt, :]
                        )
                    # sumsq via ones-matmul colsum, then ln per 512 block
                    lnrow = rows.tile([1, CHUNK], F32, tag="lnrow")
                    for b2 in range(CHUNK // 512):
                        pt = ps_cs.tile([1, 512], F32, name="cs", tag="cs")
                        for kt in range(KT):
                            nc.tensor.matmul(
                                pt,
                                onesb,
                                sq[:, kt, b2 * 512 : (b2 + 1) * 512],
                                start=(kt == 0),
                                stop=(kt == KT - 1),
                            )
                        nc.scalar.activation(
                            out=lnrow[:, b2 * 512 : (b2 + 1) * 512],
                            in_=pt,
                            func=LNF,
                        )
                    # rinv = exp(-0.5 * ln(sumsq)), rounded to f32r
                    rrow = rows.tile([1, CHUNK], F32R, tag="rrow")
                    nc.scalar.activation(out=rrow, in_=lnrow, func=EXPF, scale=-0.5)
                    # broadcast rinv across partitions via K=1 ones matmul,
                    # then zt = xt * rinv quantized to fp8
                    for b2 in range(CHUNK // 512):
                        bc = ps_cs.tile([128, 512], F32, name=f"bc{c}_{b2}", tag="cs")
                        nc.tensor.matmul(
                            bc,
                            ones1,
                            rrow[:, b2 * 512 : (b2 + 1) * 512],
                            start=True,
                            stop=True,
                        )
                        for kt in range(KT):
                            nc.vector.tensor_mul(
                                out=zt[:, kt, c0 + b2 * 512 : c0 + (b2 + 1) * 512],
                                in0=xc[:, kt, b2 * 512 : (b2 + 1) * 512],
                                in1=bc,
                            )

                sumpos = misc.tile([1, 1], F32)

                def positives():
                    # pos = colsum(z_slab * z_partner); needs chunks 0 and 4
                    pp = [ps_cs.tile([1, 512], F32, name=f"pp{j}", tag="cs") for j in range(2)]
                    for kt in range(KT):
                        pr = sqp.tile([128, CHUNK], BF16, tag="sq")
                        nc.vector.tensor_mul(
                            out=pr,
                            in0=zt[:, kt, 0:CHUNK],
                            in1=zt[:, kt, N // 2 : N // 2 + CHUNK],
                        )
                        for b2 in range(2):
                            nc.tensor.matmul(
                                pp[b2],
                                onesb,
                                pr[:, b2 * 512 : (b2 + 1) * 512],
                                start=(kt == 0),
                                stop=(kt == KT - 1),
                            )
                    posrow = misc.tile([1, CHUNK], F32)
                    for b2 in range(2):
                        nc.vector.tensor_copy(
                            out=posrow[:, b2 * 512 : (b2 + 1) * 512], in_=pp[b2]
                        )
                    nc.vector.tensor_reduce(
                        out=sumpos,
                        in_=posrow,
                        axis=mybir.AxisListType.X,
                        op=mybir.AluOpType.add,
                    )

                def gram_exp(g):
                    """fp8 DoubleRow Gram on chunk g columns + fused exp/rowsum."""
                    g0 = g * CHUNK
                    for mt in range(MT):
                        m0 = mt * 128
                        pt = ps_main.tile([128, CHUNK], F32, name=f"pt_m{g}_{mt}", tag="ptm")
                        for b2 in range(CHUNK // 512):
                            nc.tensor.matmul(
                                pt[:, b2 * 512 : (b2 + 1) * 512],
                                zt[:, 0:2, m0 : m0 + 128],
                                zt[:, 0:2, g0 + b2 * 512 : g0 + (b2 + 1) * 512],
                                start=True,
                                stop=False,
                                perf_mode=DR,
                            )
                            nc.tensor.matmul(
                                pt[:, b2 * 512 : (b2 + 1) * 512],
                                zt[:, 2:4, m0 : m0 + 128],
                                zt[:, 2:4, g0 + b2 * 512 : g0 + (b2 + 1) * 512],
                                start=False,
                                stop=True,
                                perf_mode=DR,
                            )
                        if mt in DVE_EXP_MT:
                            ei = eip.tile([128, CHUNK], I32, name=f"ei_m{g}_{mt}", tag="ei")
                            nc.vector.tensor_scalar(
                                out=ei,
                                in0=pt,
                                scalar1=A_SCH,
                                scalar2=B_SCH,
                                op0=mybir.AluOpType.mult,
                                op1=mybir.AluOpType.add,
                            )
                            nc.vector.tensor_reduce(
                                out=dparts[:, mt * NG + g : mt * NG + g + 1],
                                in_=ei.bitcast(F32),
                                axis=mybir.AxisListType.X,
                                op=mybir.AluOpType.add,
                            )
                        else:
                            et = expo.tile([128, CHUNK], F8, name=f"et_m{g}_{mt}", tag="etm")
                            nc.scalar.activation(
                                out=et,
                                in_=pt,
                                func=EXPF,
                                scale=INV_T,
                                accum_out=dparts[:, mt * NG + g : mt * NG + g + 1],
                            )

                # software pipeline: normalize runs 2 chunks ahead of the
                # Gram/exp consumer so the ScalarE queue never stalls on the
                # ln/exp -> broadcast -> scale -> matmul chain.
                normalize(0)
                normalize(1)
                for g in range(NG):
                    if g + 2 < NCH:
                        normalize(g + 2)
                    gram_exp(g)
                    if g == 4:
                        positives()


                # ---- denominators -> ln -> total ----
                denom = misc.tile([128, MT], F32)
                nc.vector.tensor_reduce(
                    out=denom,
                    in_=dparts.rearrange("p (mt g) -> p mt g", g=NG),
                    axis=mybir.AxisListType.X,
                    op=mybir.AluOpType.add,
                )
                negd = misc.tile([128, 1], F32)
                nc.vector.memset(negd, -E_DIAG)
                lnden = misc.tile([128, MT], F32R)
                nc.scalar.activation(
                    out=lnden, in_=denom, func=LNF, bias=negd[:, :], scale=1.0
                )
                lsum_ps = ps_cs.tile([1, MT], F32, name="lsum", tag="cs")
                nc.tensor.matmul(lsum_ps, ones, lnden, start=True, stop=True)
                sumln = misc.tile([1, 1], F32)
                nc.vector.tensor_reduce(
                    out=sumln,
                    in_=lsum_ps,
                    axis=mybir.AxisListType.X,
                    op=mybir.AluOpType.add,
                )

                out_sb = misc.tile([1, 2], F32)
                nc.vector.tensor_copy(out=out_sb[:, 0:1], in_=sumln)
                nc.vector.tensor_copy(out=out_sb[:, 1:2], in_=sumpos)
                nc.sync.dma_start(out=out_d[:, :], in_=out_sb)

            if loop_k:
                with tc.For_i(0, loop_k, 1):
                    body()
            else:
                body()

    # Restrict the activation-table chooser to the single table holding
    # both Exp and Ln so the scalar engine never reloads tables mid-loop.
    import bass_rust as _br
    from concourse.hw_specs import get_activation_tables as _gat

    def _single_table_loads():
        # Keep list positions (act_func_set_id is the index into
        # act_info.json) but blank out every table except the one holding
        # both Exp and Ln, so the chooser can only pick that one.
        tables = [
            (k, (v if k == "natural_log_exp_and_others" else set()))
            for k, v in _gat(nc.m.arch).items()
        ]
        assert any(v for _, v in tables), "missing natural_log_exp_and_others"
        _br.insert_act_table_loads(nc, tables)

    nc.insert_act_table_loads = _single_table_loads
    nc.compile()
    return nc


class _SpmdRunner:
    """Reusable PJRT runner (mirrors concourse.bass2jax.run_bass_via_pjrt but
    keeps the jitted executable and device-resident inputs across calls)."""

    def __init__(self, nc, n_cores):
        import jax
        from jax.sharding import Mesh, NamedSharding, PartitionSpec

        from concourse import mybir
        from concourse.bass2jax import (
            _bass_exec_p,
            install_neuronx_cc_hook,
            partition_id_tensor,
        )

        try:
            from jax.experimental.shard_map import shard_map
        except ImportError:
            from jax.shard_map import shard_map

        install_neuronx_cc_hook()
        self.jax = jax
        self.n_cores = n_cores
        partition_name = (
            nc.partition_id_tensor.name if nc.partition_id_tensor else None
        )
        in_names, out_names, out_avals, zero_outs = [], [], [], []
        for alloc in nc.m.functions[0].allocations:
            if not isinstance(alloc, mybir.MemoryLocationSet):
                continue
            name = alloc.memorylocations[0].name
            if alloc.kind == "ExternalInput":
                if name != partition_name:
                    in_names.append(name)
            elif alloc.kind == "ExternalOutput":
                shape = tuple(alloc.tensor_shape)
                dtype = mybir.dt.np(alloc.dtype)
                out_names.append(name)
                out_avals.append(jax.core.ShapedArray(shape, dtype))
                zero_outs.append(np.zeros(shape, dtype))
        self.in_names = in_names
        self.out_names = out_names
        self.zero_outs = zero_outs
        n_params = len(in_names)
        all_in = list(in_names) + list(out_names)
        if partition_name is not None:
            all_in.append(partition_name)
        donate = tuple(range(n_params, n_params + len(out_names)))

        def _body(*args):
            operands = list(args)
            if partition_name is not None:
                operands.append(partition_id_tensor())
            return tuple(
                _bass_exec_p.bind(
                    *operands,
                    out_avals=tuple(out_avals),
                    in_names=tuple(all_in),
                    out_names=tuple(out_names),
                    lowering_input_output_aliases=(),
                    sim_require_finite=True,
                    sim_require_nnan=True,
                    nc=nc,
                )
            )

        devices = jax.devices()[:n_cores]
        assert len(devices) == n_cores, (
            f"need {n_cores} neuron cores, found {len(jax.devices())}"
        )
        mesh = Mesh(np.asarray(devices), ("core",))
        n_tot = n_params + len(out_names)
        self.fn = jax.jit(
            shard_map(
                _body,
                mesh=mesh,
                in_specs=(PartitionSpec("core"),) * n_tot,
                out_specs=(PartitionSpec("core"),) * len(out_names),
                check_rep=False,
            ),
            donate_argnums=donate,
            keep_unused=True,
        )
        self.sharding = NamedSharding(mesh, PartitionSpec("core"))

    def put_inputs(self, in_maps):
        return [
            self.jax.device_put(
                np.concatenate([np.asarray(m[n]) for m in in_maps], axis=0),
                self.sharding,
            )
            for n in self.in_names
        ]

    def run(self, dev_in):
        import time

        zouts = [
            self.jax.device_put(
                np.zeros((self.n_cores * z.shape[0], *z.shape[1:]), z.dtype),
                self.sharding,
            )
            for z in self.zero_outs
        ]
        t0 = time.perf_counter()
        outs = self.fn(*dev_in, *zouts)
        for o in outs:
            o.block_until_ready()
        dt = time.perf_counter() - t0
        per_core = [dict() for _ in range(self.n_cores)]
        for i, name in enumerate(self.out_names):
            full = np.asarray(outs[i])
            rows = full.shape[0] // self.n_cores
            for c in range(self.n_cores):
                per_core[c][name] = full[c * rows : (c + 1) * rows]
        return per_core, dt


def _make_in_maps(x_i, x_j):
    import ml_dtypes

    x = np.concatenate(
        [np.asarray(x_i, np.float32), np.asarray(x_j, np.float32)], axis=0
    )
    xt = np.ascontiguousarray(x.T)  # [D, N]
    ones = np.ones((128, 1), np.float32)
    ones1 = np.ones((1, 128), np.float32)
    onesb = np.ones((128, 1), ml_dtypes.bfloat16)
    in_maps = []
    for c in range(N_CORES):
        in_maps.append(
            {
                "xt": np.ascontiguousarray(
                    np.roll(xt, -SLAB * c, axis=1)
                ).astype(ml_dtypes.bfloat16),
                "ones": ones,
                "ones1": ones1,
                "onesb": onesb,
            }
        )
    return in_maps


def _combine(per_core):
    total = 0.0
    for c in range(N_CORES):
        v = per_core[c]["out"][0]
        total += float(v[0]) - INV_T * float(v[1])
    return np.asarray(total / N, dtype=np.float32)[()]


def kernel(x_i, x_j):
    if "runner" not in _CACHE:
        nc = _build_nc()
        _CACHE["runner"] = _SpmdRunner(nc, N_CORES)
    r = _CACHE["runner"]
    per_core, _ = r.run(r.put_inputs(_make_in_maps(x_i, x_j)))
    return _combine(per_core)
